# revision 15
# baseline (speedup 1.0000x reference)
"""Trainium2 Bass kernel for the PhaseODEFunc problem.

Math (reference):
    diff = phi[:,None,:] - xi[None,:,:]                 # (B, P, N)
    m = cos(diff).sum(-1)                               # (B, P)
    w = softmax(BETA * m / N, axis=-1)                  # (B, P)
    coupling = einsum('bp,bpn->bn', w, sin(diff))       # (B, N)
    dphi = K*coupling + A*sin(omega*t - phi)            # (B, N)

Implementation: angle-addition identities turn the O(B*P*N) trig work into
4 small matmuls plus O((B+P)*N) sin/cos evaluations:
    m  = cos(phi) @ cos(xi).T + sin(phi) @ sin(xi).T
    coupling = sin(phi) . (w @ cos(xi)) - cos(phi) . (w @ sin(xi))
    sin(wt - phi) = sin(wt)cos(phi) - cos(wt)sin(phi)
so  dphi = sin(phi).(wc - A cos(wt)) - cos(phi).(ws - A sin(wt))
with wc = w @ cos(xi), ws = w @ sin(xi).

Sharding: the m/softmax stage (tiny) is replicated on all 8 cores; the
output columns (N=4096) are sharded 512 per core.  No collectives.
"""

import math

import numpy as np

import concourse.bass as bass
import concourse.bacc as bacc
import concourse.tile as tile
import concourse.mybir as mybir
import concourse.bass_utils as bass_utils
from concourse.bass import ts, ds
from concourse.masks import make_identity

TWO_PI = 2.0 * math.pi
BETA = 1.0
K_COUP = 1.0
A_ANC = 0.08
OMEGA_ANC = TWO_PI * 200.0
B, P, N = 64, 256, 4096

N_CORES = 8
NLOC = N // N_CORES          # 512 output columns per core
NCH = 128                    # contraction chunk (partition dim)
NCHUNKS = N // NCH           # 32
GRP = 3                      # chunks per PSUM staging group (3 banks)

F32 = mybir.dt.float32
F32R = mybir.dt.float32r

_cached = {}


def _build(trace_sim=False):
    nc = bacc.Bacc("TRN2", target_bir_lowering=False, debug=False,
                   num_devices=N_CORES)
    phi_d = nc.dram_tensor("phi", [B, N], F32, kind="ExternalInput")
    xi_d = nc.dram_tensor("xi", [P, N], F32, kind="ExternalInput")
    phil_d = nc.dram_tensor("phi_l", [B, NLOC], F32, kind="ExternalInput")
    xil_d = nc.dram_tensor("xi_l", [P, NLOC], F32, kind="ExternalInput")
    anc_d = nc.dram_tensor("anc", [1, 2], F32, kind="ExternalInput")
    out_d = nc.dram_tensor("dphi", [B, NLOC], F32, kind="ExternalOutput")

    from contextlib import ExitStack
    with tile.TileContext(nc, trace_sim=trace_sim) as tc:
        with ExitStack() as ctx:
            _emit(tc, phi_d.ap(), xi_d.ap(), phil_d.ap(), xil_d.ap(),
                  anc_d.ap(), out_d.ap(), ctx)
    nc.compile()
    return nc


def _emit(tc, phi, xi, phi_l, xi_l, anc, out, ctx):
    nc = tc.nc
    SIN = mybir.ActivationFunctionType.Sin
    EXP = mybir.ActivationFunctionType.Exp
    HALF_PI = math.pi / 2.0

    consts = ctx.enter_context(tc.tile_pool(name="consts", bufs=1))
    ident = consts.tile([128, 128], F32)
    make_identity(nc, ident)
    anc_sb = consts.tile([B, 2], F32)
    nc.sync.dma_start(out=anc_sb, in_=anc.to_broadcast((B, 2)))
    negpi = consts.tile([128, 1], F32)
    nc.vector.memset(negpi, -math.pi)

    sb = ctx.enter_context(tc.tile_pool(name="sb", bufs=1))

    # ---- local (output-slice) inputs + natural-layout trig --------------
    xi_l_r = xi_l.rearrange("(t p) n -> p t n", p=128)       # [128, 2, 512]
    xil_sb = sb.tile([128, 2, NLOC], F32)
    nc.sync.dma_start(out=xil_sb, in_=xi_l_r)
    phil_sb = sb.tile([B, NLOC], F32)
    nc.sync.dma_start(out=phil_sb, in_=phi_l)

    # nsxi_l/nsphi_l hold sin(x - pi) = -sin(x); signs cancel downstream.
    cxi_l = sb.tile([128, 2, NLOC], F32R)
    nsxi_l = sb.tile([128, 2, NLOC], F32R)
    nc.scalar.activation(nsxi_l, xil_sb, SIN, bias=negpi)
    xil_w = sb.tile([128, 2, NLOC], F32)
    nc.vector.add_range_wrap(xil_w, xil_sb, HALF_PI, math.pi, TWO_PI)
    nc.scalar.activation(cxi_l, xil_w, SIN)
    cphi_l = sb.tile([B, NLOC], F32)
    nsphi_l = sb.tile([B, NLOC], F32)
    nc.scalar.activation(nsphi_l, phil_sb, SIN, bias=negpi[:B])
    phil_w = sb.tile([B, NLOC], F32)
    nc.vector.add_range_wrap(phil_w, phil_sb, HALF_PI, math.pi, TWO_PI)
    nc.scalar.activation(cphi_l, phil_w, SIN)

    # ---- full inputs (replicated) --------------------------------------
    xi_r = xi.rearrange("(t p) n -> p t n", p=128)           # [128, 2, 4096]
    xi_sb = sb.tile([128, 2, N], F32)
    phi_sb = sb.tile([B, N], F32)
    for g in range(8):
        gsl = ts(g, N // 8)
        nc.sync.dma_start(out=xi_sb[:, :, gsl], in_=xi_r[:, :, gsl])
        nc.sync.dma_start(out=phi_sb[:, gsl], in_=phi[:, gsl])

    # transposed-layout trig: per chunk k the staging column block holds
    # [ xiT(p=0:128) | xiT(p=128:256) | phiT ] = 320 columns
    trigT_c = sb.tile([128, NCHUNKS, 320], F32R)
    trigT_s = sb.tile([128, NCHUNKS, 320], F32R)
    wrap_sb = sb.tile([128, NCHUNKS, 320], F32)

    mm_ps = ctx.enter_context(tc.tile_pool(name="mm_ps", bufs=1, space="PSUM"))
    m_ps = mm_ps.tile([B, P], F32)

    groups = [list(range(s0, min(s0 + GRP, NCHUNKS)))
              for s0 in range(0, NCHUNKS, GRP)]
    with tc.tile_pool(name="stage_ps", bufs=2, space="PSUM") as stage_ps:
        for chunks in groups:
            g0, ng = chunks[0], len(chunks)
            # each chunk block padded to 512 psum columns = one 2KB bank so
            # every transpose write stays inside a single bank
            stg = stage_ps.tile([128, GRP, 512], F32, name="stg")
            for j, k in enumerate(chunks):
                nc.tensor.transpose(stg[:, j, 0:128],
                                    xi_sb[:, 0, ts(k, NCH)], ident)
                nc.tensor.transpose(stg[:, j, 128:256],
                                    xi_sb[:, 1, ts(k, NCH)], ident)
                nc.tensor.transpose(stg[:, j, 256:320],
                                    phi_sb[:, ts(k, NCH)], ident[:B, :B])
            gs = ds(g0, ng)
            stg_v = stg[:, 0:ng, 0:320]
            nc.scalar.activation(trigT_s[:, gs, :], stg_v, SIN, bias=negpi)
            nc.vector.add_range_wrap(wrap_sb[:, gs, :], stg_v,
                                     HALF_PI, math.pi, TWO_PI)
            nc.scalar.activation(trigT_c[:, gs, :], wrap_sb[:, gs, :], SIN)
            for j, k in enumerate(chunks):
                nc.tensor.matmul(
                    m_ps,
                    lhsT=trigT_c[:, k, 256:320],
                    rhs=trigT_c[:, k, 0:256],
                    start=(k == 0), stop=False,
                    skip_group_check=True)
                nc.tensor.matmul(
                    m_ps,
                    lhsT=trigT_s[:, k, 256:320],
                    rhs=trigT_s[:, k, 0:256],
                    start=False, stop=(k == NCHUNKS - 1),
                    skip_group_check=True)

    # ---- softmax over p (replicated; logits = m / N, no max needed:
    # |m|/N <= 1 so exp is safe and matches softmax exactly after norm) ---
    wexp = sb.tile([B, P], F32)
    ssum = sb.tile([B, 1], F32)
    nc.scalar.activation(wexp, m_ps, EXP, scale=BETA / N, accum_out=ssum)
    rinv = sb.tile([B, 1], F32)
    nc.vector.reciprocal(rinv, ssum)
    w_sb = sb.tile([B, P], F32)
    nc.vector.tensor_scalar_mul(w_sb, in0=wexp, scalar1=rinv)

    with tc.tile_pool(name="tail_ps", bufs=1, space="PSUM") as tail_ps:
        wt_ps = tail_ps.tile([128, 2, B], F32)
        for h in range(2):
            nc.tensor.transpose(wt_ps[:, h, :], w_sb[:, ts(h, 128)],
                                ident[:B, :B])
        wT = sb.tile([128, 2, B], F32R)
        nc.vector.tensor_copy(wT, wt_ps)

        # ---- coupling on the local slice: wc = w @ cxi, ws = w @ sxi ----
        wc_ps = tail_ps.tile([B, NLOC], F32)
        ws_ps = tail_ps.tile([B, NLOC], F32)
        for h in range(2):
            nc.tensor.matmul(wc_ps,
                             lhsT=wT[:, h, :],
                             rhs=cxi_l[:, h, :],
                             start=(h == 0), stop=(h == 1),
                             skip_group_check=True)
            nc.tensor.matmul(ws_ps,
                             lhsT=wT[:, h, :],
                             rhs=nsxi_l[:, h, :],
                             start=(h == 0), stop=(h == 1),
                             skip_group_check=True)

        # dphi = sphi*(wc - A cos(wt)) - cphi*(ws - A sin(wt))
        q1 = sb.tile([B, NLOC], F32)
        q2 = sb.tile([B, NLOC], F32)
        # q1 = (wc - A cos u) * (-sin phi);  ws_ps holds -ws, so
        # q2 = (-ws + A sin u) * cos phi;  dphi = q2 - q1
        nc.vector.scalar_tensor_tensor(
            q1, in0=wc_ps, scalar=anc_sb[:, 1:2], in1=nsphi_l,
            op0=mybir.AluOpType.subtract, op1=mybir.AluOpType.mult)
        nc.vector.scalar_tensor_tensor(
            q2, in0=ws_ps, scalar=anc_sb[:, 0:1], in1=cphi_l,
            op0=mybir.AluOpType.add, op1=mybir.AluOpType.mult)
        dphi_sb = sb.tile([B, NLOC], F32)
        nc.vector.tensor_sub(dphi_sb, q2, q1)
        nc.sync.dma_start(out=out, in_=dphi_sb)


def kernel(t, phi, xi):
    t = np.asarray(t, dtype=np.float32)
    phi = np.ascontiguousarray(np.asarray(phi, dtype=np.float32))
    xi = np.ascontiguousarray(np.asarray(xi, dtype=np.float32))

    if "nc" not in _cached:
        _cached["nc"] = _build()
    nc = _cached["nc"]

    # anchor phase: match reference f32 rounding of omega*t, then take
    # sin/cos at f64 accuracy
    u = np.float32(OMEGA_ANC) * np.float32(t.reshape(-1)[0])
    anc = np.array([[A_ANC * math.sin(float(u)),
                     A_ANC * math.cos(float(u))]], dtype=np.float32)

    in_maps = []
    for c in range(N_CORES):
        sl = slice(c * NLOC, (c + 1) * NLOC)
        in_maps.append({
            "phi": phi,
            "xi": xi,
            "phi_l": np.ascontiguousarray(phi[:, sl]),
            "xi_l": np.ascontiguousarray(xi[:, sl]),
            "anc": anc,
        })

    res = bass_utils.run_bass_kernel_spmd(
        nc, in_maps, core_ids=list(range(N_CORES)))
    return np.concatenate([res.results[c]["dphi"] for c in range(N_CORES)],
                          axis=1)
